# revision 1
# baseline (speedup 1.0000x reference)
"""Trainium2 Bass kernel for nn_Attention_15109694947883.

Causal self-attention where (due to the reference's source quirk) q, k, v
all come from the first third of the qkv projection, so only
w_qkv[:, :1024] participates.

Sharding: head-parallel across 8 cores. Core c handles heads (2c, 2c+1)
for both batches: it gets w_qkv columns [128c, 128c+128) and w_out rows
[128c, 128c+128), computes its partial output [4096, 1024]; the host sums
the 8 partials and adds the bias.

Self-contained: only needs numpy + jax + the concourse stack that the
runtime environment provides.
"""

import numpy as np

# Problem constants (hardcoded per harness contract)
B = 2
SEQ = 2048
DIM = 1024
HEADS = 16
DH = 64
SCALE = DH ** -0.5
N_CORES = 8
HD = 128          # head dims per core = 2 heads x 64
CB = 128          # key/col block
RB = 512          # row group block
NEG = -1.0e9      # additive causal mask value (pre-scale)


def _split_waits(nc, mybir, maxw=1):
    """This walrus build rejects >maxw sync waits on one instruction
    (seen on Tile's tail drain). Split excess waits onto preceding
    same-engine NoOps — engines execute their stream in order, so the
    blocking semantics are identical."""
    n = 0
    for f in nc.m.functions:
        for bb in f.blocks:
            insts = list(bb.instructions)
            out = []
            for inst in insts:
                si = inst.sync_info
                if si is not None and si.on_wait and len(si.on_wait) > maxw:
                    waits = list(si.on_wait)
                    head, rest = waits[:-maxw], waits[-maxw:]
                    while head:
                        chunk, head = head[:maxw], head[maxw:]
                        nop = mybir.InstNoOp(
                            name=f"I-waitsplit-{nc.next_id()}", ins=[], outs=[]
                        )
                        nop.engine = inst.engine
                        nop.sync_info = mybir.SyncInfo(
                            on_wait=chunk, on_update=[]
                        )
                        out.append(nop)
                        n += 1
                    si.on_wait = rest
                out.append(inst)
            if len(out) != len(insts):
                bb.instructions = out
    return n


def build_nc(seq=SEQ, dim=DIM, b=B, mm_dt="f32r", pv_dt="bf16", loop_r=0, x_dt=None, no_denom=False):
    """Build the per-core SPMD Bass program.

    mm_dt: dtype for scores / proj / out-proj matmuls ("f32r" or "f32")
    pv_dt: dtype for P^T and the PV + denominator matmuls ("bf16"/"f32r"/"f32")
    """
    from contextlib import ExitStack

    import concourse.bass as bass
    import concourse.mybir as mybir
    import concourse.tile as tile
    from concourse.masks import make_identity

    f32 = mybir.dt.float32
    f32r = mybir.dt.float32r
    bf16 = mybir.dt.bfloat16
    mmd = {"f32r": f32r, "f32": f32}[mm_dt]
    xd = {None: mmd, "bf16": bf16, "f32r": f32r, "f32": f32}[x_dt]
    pvd = {"bf16": bf16, "f32r": f32r, "f32": f32}[pv_dt]
    pv_store = bf16 if pv_dt == "bf16" else f32  # storage dtype of P^T / QN

    nb = b * seq            # total rows
    kt = dim // 128         # contraction tiles for projection
    nblk = nb // RB         # 512-row blocks for projection
    jcs = seq // CB         # key blocks per batch
    gs = seq // RB          # row groups per batch

    nc = bass.Bass("TRN2", target_bir_lowering=False, debug=False)
    xT = nc.dram_tensor("xT", [dim, nb], xd, kind="ExternalInput").ap()
    w1 = nc.dram_tensor("w1", [dim, HD], xd, kind="ExternalInput").ap()
    w2 = nc.dram_tensor("w2", [HD, dim], mmd, kind="ExternalInput").ap()
    y = nc.dram_tensor("y", [nb, dim], f32, kind="ExternalOutput").ap()
    itc = (nc.dram_tensor("itc", [1, 1], f32, kind="ExternalOutput").ap()
           if loop_r > 0 else None)

    def mm(out, lhsT, rhs, dt, **kw):
        nc.tensor.matmul(out, lhsT, rhs, **kw)

    with tile.TileContext(nc) as tc, ExitStack() as ctx:
        cpool = ctx.enter_context(tc.tile_pool(name="consts", bufs=1))
        ident = cpool.tile([128, 128], f32, tag="ident")
        make_identity(nc, ident[:])
        # cmask[c, rr] = 0 if rr >= c else NEG   (S^T diag-block causal mask)
        cmask = cpool.tile([128, 128], f32, tag="cmask")
        nc.gpsimd.memset(cmask[:], 0.0)
        nc.gpsimd.affine_select(
            out=cmask[:], in_=cmask[:],
            compare_op=mybir.AluOpType.is_ge, fill=NEG,
            base=0, pattern=[[1, 128]], channel_multiplier=-1,
        )
        ones = cpool.tile([128, 1], pv_store, tag="ones")
        nc.gpsimd.memset(ones[:], 1.0)

        wpool = ctx.enter_context(tc.tile_pool(name="w", bufs=1))
        W1 = wpool.tile([128, kt * HD], xd, tag="w1")
        for k in range(kt):
            nc.sync.dma_start(W1[:, k * HD:(k + 1) * HD],
                              w1[k * 128:(k + 1) * 128, :])
        W2 = wpool.tile([128, dim], mmd, tag="w2")

        qpool = ctx.enter_context(tc.tile_pool(name="q", bufs=1))
        QT = qpool.tile([128, nb], mmd, tag="qt")      # [head-dim, row]
        QN = qpool.tile([128, nb], pv_store, tag="qn")  # [row-in-block, head-dim]

        psum = ctx.enter_context(tc.tile_pool(name="ps", bufs=1, space="PSUM"))
        ptpool = ctx.enter_context(tc.tile_pool(name="pt", bufs=1))
        spool = ctx.enter_context(tc.tile_pool(name="sm", bufs=2))
        onpool = ctx.enter_context(tc.tile_pool(name="on", bufs=2))
        ypool = ctx.enter_context(tc.tile_pool(name="ysb", bufs=3))

        xpool = ctx.enter_context(tc.tile_pool(name="xt", bufs=2))
        if loop_r > 0:
            itile = cpool.tile([1, 1], f32, tag="itile")
            nc.gpsimd.memset(itile[:], 0.0)
            nc.sync.dma_start(W2[:], w2[:, :])

        # ---- Phase 1: q projection (QT = w1^T @ x^T) + transposes (QN) ----
        from contextlib import nullcontext
        loop_ctx = ExitStack()
        if loop_r > 0:
            loop_ctx.enter_context(tc.For_i(0, loop_r, 1))
            it2 = cpool.tile([1, 1], f32, tag="it2")
            nc.scalar.add(it2[:], itile[:], 1.0)
            nc.vector.tensor_copy(itile[:], it2[:])
        if True:
            for blk in range(nblk):
                xcols = []
                for k in range(kt):
                    xc = xpool.tile([128, RB], xd, tag=f"xt{k}", bufs=2)
                    nc.sync.dma_start(
                        xc[:],
                        xT[k * 128:(k + 1) * 128, blk * RB:(blk + 1) * RB])
                    xcols.append(xc)
                qps = psum.tile([128, RB], f32, tag="pa", bufs=2)
                for k in range(kt):
                    mm(qps[:], W1[:, k * HD:(k + 1) * HD], xcols[k][:], mmd,
                       start=(k == 0), stop=(k == kt - 1))
                nc.vector.tensor_copy(QT[:, blk * RB:(blk + 1) * RB], qps[:])
                if blk == 0 and loop_r == 0:
                    nc.sync.dma_start(W2[:], w2[:, :])
                for s in range(RB // 128):
                    col = blk * RB + s * 128
                    tps = psum.tile([128, 128], f32, tag="pb", bufs=2,
                                    padded_shape=[128, RB])
                    nc.tensor.transpose(tps[:], QT[:, col:col + 128].bitcast(f32), ident[:])
                    nc.vector.tensor_copy(QN[:, col:col + 128], tps[:])

        # ---- Phase 2: attention per batch ----
        # Emission order interleaves the two batches: b1's first score
        # blocks are emitted before b0's PV phase so the scheduler keeps
        # ScalarE's exp pipeline fed while the PE runs b0's PV matmuls.
        PTs = [dict() for _ in range(b)]

        def emit_scores(bi, jc_lo, jc_hi, desc=False):
            base = bi * seq
            PT = PTs[bi]
            order = range(jc_hi - 1, jc_lo - 1, -1) if desc else                 range(jc_lo, jc_hi)
            for jc in order:
                r0 = CB * jc             # first valid query col (absolute r)
                cw = seq - r0
                for h in range(2):
                    pt = ptpool.tile([128, cw], pv_store, tag=f"pt{h}_{jc}",
                                     bufs=(2 if jc < 4 else 1))
                    PT[(h, jc)] = pt
                    lhsT = QT[64 * h:64 * h + 64, base + r0:base + r0 + 128]
                    jr = r0 // RB
                    while jr * RB < seq:
                        cs = max(RB * jr, r0)
                        ce = RB * (jr + 1)
                        w = ce - cs
                        sp = psum.tile([128, RB], f32, tag="sp", bufs=3)
                        mm(sp[:, 0:w], lhsT,
                           QT[64 * h:64 * h + 64, base + cs:base + ce],
                           mmd, start=True, stop=True,
                           tile_position=(64 * h, 0))
                        if cs == r0:  # diag block sits at chunk start
                            nc.vector.tensor_add(
                                sp[:, 0:128], sp[:, 0:128], cmask[:])
                        nc.scalar.activation(
                            pt[:, cs - r0:ce - r0], sp[:, 0:w],
                            mybir.ActivationFunctionType.Exp,
                            bias=0.0, scale=float(SCALE))
                        jr += 1

        def emit_pv(bi, desc=False):
            base = bi * seq
            PT = PTs[bi]
            # -- PV + denominators + normalize + out-projection, per group --
            g_order = list(reversed(range(gs))) if desc else list(range(gs))
            for g in g_order:
                ops = psum.tile([128, RB], f32, tag="pa", bufs=2)
                dps = (None if no_denom else
                       psum.tile([128, RB], f32, tag="pb", bufs=2))
                njc = (g + 1) * (RB // CB)
                # denominator matmuls first: dps completes early so the
                # reciprocal chain overlaps the PV accumulation below
                if not no_denom:
                    for jc in range(njc):
                        r0 = CB * jc
                        cs = max(RB * g, r0)
                        w = RB * (g + 1) - cs
                        for h in range(2):
                            pts = PT[(h, jc)][:, cs - r0:cs - r0 + w]
                            mm(dps[32 * h:32 * h + 1,
                                   cs - RB * g:cs - RB * g + w],
                               ones[:, 0:1], pts, pvd,
                               start=(jc == 0), stop=(jc == njc - 1),
                               tile_position=(0, 32 * h),
                               skip_group_check=True)
                for jc in range(njc):
                    r0 = CB * jc
                    cs = max(RB * g, r0)
                    w = RB * (g + 1) - cs
                    for h in range(2):
                        pts = PT[(h, jc)][:, cs - r0:cs - r0 + w]
                        qn = QN[:, base + r0 + 64 * h:base + r0 + 64 * h + 64]
                        mm(ops[64 * h:64 * h + 64, cs - RB * g:cs - RB * g + w],
                           qn, pts, pvd,
                           start=(jc == 0), stop=(jc == njc - 1),
                           tile_position=(0, 64 * h), skip_group_check=True)
                on = onpool.tile([128, RB], mmd, tag="on")
                if no_denom:
                    nc.vector.tensor_copy(on[:], ops[:])
                else:
                    # denominators -> reciprocal -> broadcast tile
                    dsb = spool.tile([64, RB], f32, tag="dsb")
                    nc.vector.tensor_copy(dsb[0:1, :], dps[0:1, :])
                    nc.vector.tensor_copy(dsb[32:33, :], dps[32:33, :])
                    rg = spool.tile([16, 64], f32, tag="rg")
                    nc.sync.dma_start(rg[0:16, :], dsb[0:33:32, :])
                    rr = spool.tile([16, 64], f32, tag="rr")
                    nc.vector.reciprocal(rr[:], rg[:])
                    rt = spool.tile([2, RB], f32, tag="rt")
                    nc.sync.dma_start(rt[0:2, :], rr[0:16, :])
                    bc = spool.tile([128, RB], f32, tag="bc")
                    nc.sync.dma_start(
                        bc[0:128, :],
                        rt[0:2, :].unsqueeze(1).to_broadcast([2, 64, RB]))
                    nc.vector.tensor_mul(on[:], ops[:], bc[:])
                # out-projection for this row group
                ew = min(RB, dim)
                for rb_i in range(RB // 128):
                    for eb in range(dim // ew):
                        tail = (bi == b - 1 and g == gs - 1)
                        yps = psum.tile(
                            [128, ew], f32,
                            tag=("pb" if tail and eb % 2 == 1 else "yps"),
                            bufs=(2 if tail and eb % 2 == 1 else 1))
                        mm(yps[:], on[:, rb_i * 128:(rb_i + 1) * 128],
                           W2[:, eb * ew:(eb + 1) * ew], mmd,
                           start=True, stop=True)
                        ysb = ypool.tile([128, ew], f32, tag="ysb")
                        nc.vector.tensor_copy(ysb[:], yps[:])
                        r_out = base + RB * g + 128 * rb_i
                        nc.sync.dma_start(
                            y[r_out:r_out + 128, eb * ew:(eb + 1) * ew], ysb[:])

        nhead = min(4, jcs)
        emit_scores(0, 0, jcs)
        if b > 1:
            emit_scores(1, 0, nhead)
        emit_pv(0)
        if b > 1:
            emit_scores(1, nhead, jcs)
            emit_pv(1)

        loop_ctx.close()
        if loop_r > 0:
            nc.sync.dma_start(itc[:], itile[:])

    return nc


# ---------------------------------------------------------------------------
# Host wrapper
# ---------------------------------------------------------------------------

_CACHE = {}


def _get_nc():
    if "nc" not in _CACHE:
        import concourse.mybir as mybir
        nc = build_nc()
        _split_waits(nc, mybir, maxw=1)
        _CACHE["nc"] = nc
    return _CACHE["nc"]


def make_in_maps(x, w_qkv):
    xf = np.ascontiguousarray(x.reshape(B * SEQ, DIM))
    xT = np.ascontiguousarray(xf.T)
    in_maps = []
    for c in range(N_CORES):
        in_maps.append({
            "xT": xT,
            "w1": np.ascontiguousarray(w_qkv[:, HD * c:HD * (c + 1)]),
            "w2": None,  # filled by caller (needs w_out)
        })
    return in_maps


def kernel(x, w_qkv, w_out, b_out):
    import jax
    jax.devices()  # ensure axon backend initialized
    from concourse.bass_utils import run_bass_kernel_spmd

    nc = _get_nc()
    xf = np.ascontiguousarray(np.asarray(x, dtype=np.float32).reshape(B * SEQ, DIM))
    xT = np.ascontiguousarray(xf.T)
    w_qkv = np.asarray(w_qkv, dtype=np.float32)
    w_out = np.asarray(w_out, dtype=np.float32)
    in_maps = [
        {
            "xT": xT,
            "w1": np.ascontiguousarray(w_qkv[:, HD * c:HD * (c + 1)]),
            "w2": np.ascontiguousarray(w_out[HD * c:HD * (c + 1), :]),
        }
        for c in range(N_CORES)
    ]
    res = run_bass_kernel_spmd(nc, in_maps, list(range(N_CORES)))
    acc = np.zeros((B * SEQ, DIM), dtype=np.float32)
    for c in range(N_CORES):
        acc += res.results[c]["y"]
    acc += np.asarray(b_out, dtype=np.float32)[None, :]
    return acc.reshape(B, SEQ, DIM)

